# revision 11
# baseline (speedup 1.0000x reference)
"""MAGNN metapath-specific GNN message passing on 8 Trainium2 NeuronCores.

Strategy (dst-sharding, no collectives):
  - Sort edges by destination node on host; core c owns dst nodes
    [c*12500, (c+1)*12500). Each destination's softmax segment lives
    entirely on one core, so no cross-core reduction is needed.
  - Host prepares (index manipulation only): gather indices, per-slot
    local-dst ids, and the tiny (256x64 / 256x8) rotation + attention
    weight matrices A|B derived from r_vec/attn params.
  - The per-(edge,position) feature gather uses one indirect DMA per 128
    rows (the SWDGE descriptor rate, ~8-9ns/descriptor serialized on the
    Pool engine, is the hard bottleneck for this memory-regime problem;
    batched dma_gather / staging variants all cost the same or more
    descriptors, measured on HW).
  - Device, per window (<=128 dst nodes, 384 edge slots = 3 chunks):
      PE-transpose edata halves, matmul with A|B -> [hidden | a]
      leaky-relu (ACT Prelu) + exp (ACT) -> aexp (bf16)
      one-hot scatter matrix built on DVE (iota == local dst, bf16)
      rhs = hidden (x) aexp broadcast-mul (DVE) [128, 512] bf16
      U += onehot^T @ rhs ; asum += onehot^T @ aexp  (PE bf16 in, PSUM accum)
      evict: out = U * 1/max(asum,eps) fused on DVE -> bf16, DMA to DRAM
  - Host unshards the padded window-major bf16 output back to [N, 8, 64] f32.
"""

import sys

if "/opt/trn_rl_repo" not in sys.path:
    sys.path.insert(0, "/opt/trn_rl_repo")

import numpy as np

import concourse.bass as bass
import concourse.mybir as mybir
import concourse.tile as tile
from concourse import bacc
from concourse._compat import axon_active
from concourse.bass_utils import run_bass_kernel_spmd
from concourse.masks import make_identity

# Problem constants (hardcoded per spec)
ETYPES = [0, 2, 1]
D = 64
D2 = 32
H = 8
ALPHA = 0.01
R = 2
N = 100000
E = 250000
L = 4

NCORES = 8
NPC = N // NCORES  # nodes per core
P = 128
NSUB = 3  # chunks per window
EW = NSUB * P  # edge slots per window
QW = NSUB * L  # chunks-of-64 per window in the ed layout

GB = 7  # windows per ed tile


def _build_AB(r_vec, attn1_w, attn2):
    """A [256,64]: edata(rotated,meaned)->hidden.  B [256,8]: edata->attn logits."""
    rv = np.asarray(r_vec, np.float64)
    rv = rv / np.linalg.norm(rv, axis=2, keepdims=True)
    rv_conj = rv * np.array([1.0, -1.0])
    rv2 = np.stack((rv, rv_conj), axis=1).reshape(2 * R, D2, 2)
    fr = [None] * L
    fr[L - 1] = np.stack([np.ones(D2), np.zeros(D2)], -1)
    for i in range(L - 2, -1, -1):
        r = rv2[ETYPES[i]]
        re = fr[i + 1][:, 0] * r[:, 0] - fr[i + 1][:, 1] * r[:, 1]
        im = fr[i + 1][:, 0] * r[:, 1] + fr[i + 1][:, 1] * r[:, 0]
        fr[i] = np.stack([re, im], -1)
    frv = np.stack(fr, 0)  # [L, D2, 2]

    A = np.zeros((L * D, D))
    for l in range(L):
        c = frv[l, :, 0]
        s = frv[l, :, 1]
        for p in range(D2):
            A[l * D + 2 * p, 2 * p] = c[p] / L
            A[l * D + 2 * p, 2 * p + 1] = s[p] / L
            A[l * D + 2 * p + 1, 2 * p] = -s[p] / L
            A[l * D + 2 * p + 1, 2 * p + 1] = c[p] / L
    W1 = np.asarray(attn1_w, np.float64)  # [H, D]
    W2 = np.asarray(attn2, np.float64).reshape(H, D)
    B = A @ W2.T  # [256, H]
    B[(L - 1) * D : L * D, :] += W1.T  # center term (position L-1 raw feature)
    return A.astype(np.float32), B.astype(np.float32)


def _host_prep(emi, dst):
    """Sort/shard/window edges. Returns per-core arrays + window metadata."""
    emi = np.asarray(emi).astype(np.int64)
    dst = np.asarray(dst).astype(np.int64)
    deg = np.bincount(dst, minlength=N)
    assert deg.max() <= EW
    order = np.argsort(dst, kind="stable")
    dst_sorted = dst[order]

    core_wins = []  # per core: list of (node_base, span, estart, cnt)
    for c in range(NCORES):
        lo, hi = c * NPC, (c + 1) * NPC
        elo = int(np.searchsorted(dst_sorted, lo))
        wins = []
        node = lo
        eptr = elo
        while node < hi:
            base = node
            cnt = 0
            while node < hi and node - base < P:
                dn = int(deg[node])
                if cnt + dn > EW:
                    break
                cnt += dn
                node += 1
            wins.append((base, node - base, eptr, cnt))
            eptr += cnt
        core_wins.append(wins)

    NW = max(len(w) for w in core_wins)

    offs_all = []
    dstloc_all = []
    for c in range(NCORES):
        wins = core_wins[c]
        base_w = np.zeros(NW, np.int64)
        cnt_w = np.zeros(NW, np.int64)
        est_w = np.zeros(NW, np.int64)
        for i, (b, sp, es, cn) in enumerate(wins):
            base_w[i], est_w[i], cnt_w[i] = b, es, cn

        slot = np.arange(EW)  # slot = s*128 + p
        valid = slot[None, :] < cnt_w[:, None]  # [NW, EW]
        eidx = np.minimum(est_w[:, None] + slot[None, :], E - 1)
        eids = order[eidx]  # [NW, EW] edge ids (garbage where invalid)

        gidx = emi[eids].astype(np.int32)  # [NW, EW, 4]
        gidx[~valid] = 0
        # offs layout: [128 p, NW, NSUB, L]  (slot = s*128+p -> [s, p])
        offs = gidx.reshape(NW, NSUB, P, L).transpose(2, 0, 1, 3)
        offs_all.append(np.ascontiguousarray(offs, np.int32).reshape(P, NW * QW))

        dl = (dst[eids] - base_w[:, None]).astype(np.float32)  # [NW, EW]
        dl[~valid] = -1.0
        dloc = dl.reshape(NW, NSUB, P).transpose(2, 0, 1)  # [p, w, s]
        dstloc_all.append(np.ascontiguousarray(dloc, np.float32).reshape(P, NW * NSUB))

    return NW, core_wins, offs_all, dstloc_all


_CACHE = {}


def _build_program(NW):
    if NW in _CACHE:
        return _CACHE[NW]
    fp32 = mybir.dt.float32
    bf16 = mybir.dt.bfloat16
    nc = bacc.Bacc(
        "TRN2",
        target_bir_lowering=False,
        debug=not axon_active(),
        num_devices=NCORES,
    )
    feat_d = nc.dram_tensor("features", [N, D], fp32, kind="ExternalInput").ap()
    ab_d = nc.dram_tensor("ab", [P, 144], fp32, kind="ExternalInput").ap()
    offs_d = nc.dram_tensor("offs", [P, NW * QW], mybir.dt.int32, kind="ExternalInput").ap()
    dloc_d = nc.dram_tensor("dstloc", [P, NW * NSUB], fp32, kind="ExternalInput").ap()
    out_d = nc.dram_tensor("out", [NW * P, H * D], bf16, kind="ExternalOutput").ap()

    with tile.TileContext(nc) as tc:
        with (
            tc.tile_pool(name="const", bufs=1) as cpool,
            tc.tile_pool(name="oh", bufs=3) as ohpool,
            tc.tile_pool(name="ed", bufs=2) as edpool,
            tc.tile_pool(name="ts", bufs=4) as tspool,
            tc.tile_pool(name="sm", bufs=6) as smpool,
            tc.tile_pool(name="rhs", bufs=3) as rhspool,
            tc.tile_pool(name="ob", bufs=3) as obpool,
            tc.tile_pool(name="pt", bufs=3, space="PSUM") as ptpool,
            tc.tile_pool(name="pha", bufs=2, space="PSUM") as phapool,
            tc.tile_pool(name="pu", bufs=2, space="PSUM") as pupool,
            tc.tile_pool(name="pas", bufs=1, space="PSUM") as paspool,
        ):
            ident = cpool.tile([P, P], fp32, tag="ident")
            make_identity(nc, ident[:])
            iota = cpool.tile([P, P], fp32, tag="iota")
            nc.gpsimd.iota(
                iota[:], [[1, P]], channel_multiplier=0,
                allow_small_or_imprecise_dtypes=True,
            )
            ab_t = cpool.tile([P, 144], fp32, tag="ab")
            nc.sync.dma_start(out=ab_t[:], in_=ab_d[:, :])
            offs_t = cpool.tile([P, NW * QW], mybir.dt.int32, tag="offs")
            nc.sync.dma_start(out=offs_t[:], in_=offs_d[:, :])
            dloc_t = cpool.tile([P, NW * NSUB], fp32, tag="dloc")
            nc.sync.dma_start(out=dloc_t[:], in_=dloc_d[:, :])

            # ---- per-batch gather + compute pipeline
            for b in range((NW + GB - 1) // GB):
                gb = min(GB, NW - b * GB)
                ed_t = edpool.tile([P, GB * QW * D], fp32, tag="ed")
                for q in range(gb * QW):
                    nc.gpsimd.indirect_dma_start(
                        out=ed_t[:, q * D : (q + 1) * D],
                        out_offset=None,
                        in_=feat_d[:, :],
                        in_offset=bass.IndirectOffsetOnAxis(
                            ap=offs_t[:, b * GB * QW + q : b * GB * QW + q + 1],
                            axis=0,
                        ),
                    )
                for j in range(gb):
                    w = b * GB + j
                    u_p = pupool.tile([P, H * D], fp32, tag="u")
                    as_p = paspool.tile([P, H], fp32, tag="as")
                    for s in range(NSUB):
                        off = (j * QW + s * L) * D
                        ed_s = ed_t[:, off : off + L * D]  # [128, 256]
                        t0_p = ptpool.tile([P, P], fp32, tag="tp")
                        t1_p = ptpool.tile([P, P], fp32, tag="tp")
                        nc.tensor.transpose(out=t0_p[:], in_=ed_s[:, 0:P], identity=ident[:])
                        nc.tensor.transpose(out=t1_p[:], in_=ed_s[:, P : 2 * P], identity=ident[:])
                        t0_s = tspool.tile([P, P], fp32, tag="tsb")
                        t1_s = tspool.tile([P, P], fp32, tag="tsb")
                        nc.scalar.activation(out=t0_s[:], in_=t0_p[:], func=mybir.ActivationFunctionType.Copy)
                        nc.scalar.activation(out=t1_s[:], in_=t1_p[:], func=mybir.ActivationFunctionType.Copy)
                        ha_p = phapool.tile([P, 72], fp32, tag="ha")
                        nc.tensor.matmul(out=ha_p[:], lhsT=t0_s[:], rhs=ab_t[:, 0:72], start=True, stop=False)
                        nc.tensor.matmul(out=ha_p[:], lhsT=t1_s[:], rhs=ab_t[:, 72:144], start=False, stop=True)
                        # exp(leaky_relu(a)) on ACT (both in the exp act table set)
                        alr = smpool.tile([P, H], fp32, tag="alr")
                        nc.scalar.activation(
                            out=alr[:], in_=ha_p[:, D : D + H],
                            func=mybir.ActivationFunctionType.Prelu, alpha=ALPHA,
                        )
                        aexp = smpool.tile([P, H], bf16, tag="aexp")
                        nc.scalar.activation(out=aexp[:], in_=alr[:], func=mybir.ActivationFunctionType.Exp)
                        # one-hot scatter matrix [slot, node] built on DVE
                        oh_t = ohpool.tile([P, P], bf16, tag="oh")
                        nc.vector.tensor_scalar(
                            out=oh_t[:], in0=iota[:],
                            scalar1=dloc_t[:, w * NSUB + s : w * NSUB + s + 1],
                            scalar2=None,
                            op0=mybir.AluOpType.is_equal,
                        )
                        # rhs [128, 512] = hidden (h-bcast) * aexp (d-bcast)
                        rhs_t = rhspool.tile([P, H * D], bf16, tag="rhs")
                        nc.vector.tensor_tensor(
                            out=rhs_t[:].rearrange("p (h d) -> p h d", h=H),
                            in0=ha_p[:, 0:D].unsqueeze(1).to_broadcast([P, H, D]),
                            in1=aexp[:].unsqueeze(2).to_broadcast([P, H, D]),
                            op=mybir.AluOpType.mult,
                        )
                        nc.tensor.matmul(
                            out=u_p[:], lhsT=oh_t[:], rhs=rhs_t[:],
                            start=(s == 0), stop=(s == NSUB - 1),
                            skip_group_check=True,
                        )
                        nc.tensor.matmul(
                            out=as_p[:], lhsT=oh_t[:], rhs=aexp[:],
                            start=(s == 0), stop=(s == NSUB - 1),
                            skip_group_check=True,
                        )
                    # epilogue: out = U / max(asum, eps), cast to bf16
                    asafe = smpool.tile([P, H], fp32, tag="asafe")
                    nc.vector.tensor_scalar_max(out=asafe[:], in0=as_p[:], scalar1=1e-20)
                    rec = smpool.tile([P, H], fp32, tag="rec")
                    nc.vector.reciprocal(out=rec[:], in_=asafe[:])
                    o_sb = obpool.tile([P, H * D], bf16, tag="osb")
                    nc.vector.tensor_tensor(
                        out=o_sb[:].rearrange("p (h d) -> p h d", h=H),
                        in0=u_p[:].rearrange("p (h d) -> p h d", h=H),
                        in1=rec[:].unsqueeze(2).to_broadcast([P, H, D]),
                        op=mybir.AluOpType.mult,
                    )
                    nc.sync.dma_start(out=out_d[w * P : (w + 1) * P, :], in_=o_sb[:])

    nc.compile()
    _CACHE[NW] = nc
    return nc


LAST_RESULT = None


def kernel(features, r_vec, attn1_w, attn2, edge_metapath_indices, edge_dst):
    global LAST_RESULT
    features = np.ascontiguousarray(np.asarray(features, np.float32))
    A, B = _build_AB(r_vec, attn1_w, attn2)
    ab = np.concatenate([A, B], axis=1)  # [256, 72]
    # ab dram layout [128, 144]: [:, 0:72] = rows 0:128, [:, 72:144] = rows 128:256
    ab_host = np.concatenate([ab[:P], ab[P:]], axis=1).astype(np.float32)
    ab_host = np.ascontiguousarray(ab_host)

    NW, core_wins, offs_all, dstloc_all = _host_prep(
        edge_metapath_indices, edge_dst
    )
    nc = _build_program(NW)

    in_maps = []
    for c in range(NCORES):
        in_maps.append(
            {
                "features": features,
                "ab": ab_host,
                "offs": offs_all[c],
                "dstloc": dstloc_all[c],
            }
        )
    res = run_bass_kernel_spmd(nc, in_maps, core_ids=list(range(NCORES)))
    LAST_RESULT = res

    out = np.zeros((N, H, D), np.float32)
    for c in range(NCORES):
        co = np.asarray(res.results[c]["out"], dtype=np.float32)  # [NW*128, 512]
        for w, (base, span, _es, _cn) in enumerate(core_wins[c]):
            if span > 0:
                out[base : base + span] = co[w * P : w * P + span].reshape(span, H, D)
    return out


if __name__ == "__main__":
    import reference

    inputs = {k: np.asarray(v) for k, v in reference.setup_inputs().items()}
    got = kernel(**inputs)
    print("kernel output", got.shape, got.dtype)


# revision 12
# speedup vs baseline: 1.0109x; 1.0109x over previous
"""MAGNN metapath-specific GNN message passing on 8 Trainium2 NeuronCores.

Strategy (dst-sharding, no collectives):
  - Sort edges by destination node on host; core c owns dst nodes
    [c*12500, (c+1)*12500). Each destination's softmax segment lives
    entirely on one core, so no cross-core reduction is needed.
  - Host prepares (index manipulation only): gather indices, per-slot
    local-dst ids, and the tiny (256x64 / 256x8) rotation + attention
    weight matrices A|B derived from r_vec/attn params.
  - The per-(edge,position) feature gather uses one indirect DMA per 128
    rows (the SWDGE descriptor rate, ~8-9ns/descriptor serialized on the
    Pool engine, is the hard bottleneck for this memory-regime problem;
    batched dma_gather / staging variants all cost the same or more
    descriptors, measured on HW).
  - Device, per window (<=128 dst nodes, 384 edge slots = 3 chunks):
      PE-transpose edata halves, matmul with A|B -> [hidden | a]
      leaky-relu (ACT Prelu) + exp (ACT) -> aexp (bf16)
      one-hot scatter matrix built on DVE (iota == local dst, bf16)
      rhs = hidden (x) aexp broadcast-mul (DVE) [128, 512] bf16
      U += onehot^T @ rhs ; asum += onehot^T @ aexp  (PE bf16 in, PSUM accum)
      evict: out = U * 1/max(asum,eps) fused on DVE -> bf16, DMA to DRAM
  - Host unshards the padded window-major bf16 output back to [N, 8, 64] f32.
"""

import sys

if "/opt/trn_rl_repo" not in sys.path:
    sys.path.insert(0, "/opt/trn_rl_repo")

import numpy as np

import concourse.bass as bass
import concourse.mybir as mybir
import concourse.tile as tile
from concourse import bacc
from concourse._compat import axon_active
from concourse.bass_utils import run_bass_kernel_spmd
from concourse.masks import make_identity

# Problem constants (hardcoded per spec)
ETYPES = [0, 2, 1]
D = 64
D2 = 32
H = 8
ALPHA = 0.01
R = 2
N = 100000
E = 250000
L = 4

NCORES = 8
NPC = N // NCORES  # nodes per core
P = 128
NSUB = 3  # chunks per window
EW = NSUB * P  # edge slots per window
QW = NSUB * L  # chunks-of-64 per window in the ed layout

GB = 7  # windows per ed tile


def _build_AB(r_vec, attn1_w, attn2):
    """A [256,64]: edata(rotated,meaned)->hidden.  B [256,8]: edata->attn logits."""
    rv = np.asarray(r_vec, np.float64)
    rv = rv / np.linalg.norm(rv, axis=2, keepdims=True)
    rv_conj = rv * np.array([1.0, -1.0])
    rv2 = np.stack((rv, rv_conj), axis=1).reshape(2 * R, D2, 2)
    fr = [None] * L
    fr[L - 1] = np.stack([np.ones(D2), np.zeros(D2)], -1)
    for i in range(L - 2, -1, -1):
        r = rv2[ETYPES[i]]
        re = fr[i + 1][:, 0] * r[:, 0] - fr[i + 1][:, 1] * r[:, 1]
        im = fr[i + 1][:, 0] * r[:, 1] + fr[i + 1][:, 1] * r[:, 0]
        fr[i] = np.stack([re, im], -1)
    frv = np.stack(fr, 0)  # [L, D2, 2]

    A = np.zeros((L * D, D))
    for l in range(L):
        c = frv[l, :, 0]
        s = frv[l, :, 1]
        for p in range(D2):
            A[l * D + 2 * p, 2 * p] = c[p] / L
            A[l * D + 2 * p, 2 * p + 1] = s[p] / L
            A[l * D + 2 * p + 1, 2 * p] = -s[p] / L
            A[l * D + 2 * p + 1, 2 * p + 1] = c[p] / L
    W1 = np.asarray(attn1_w, np.float64)  # [H, D]
    W2 = np.asarray(attn2, np.float64).reshape(H, D)
    B = A @ W2.T  # [256, H]
    B[(L - 1) * D : L * D, :] += W1.T  # center term (position L-1 raw feature)
    return A.astype(np.float32), B.astype(np.float32)


def _host_prep(emi, dst):
    """Sort/shard/window edges. Returns per-core arrays + window metadata."""
    emi = np.asarray(emi).astype(np.int64)
    dst = np.asarray(dst).astype(np.int64)
    deg = np.bincount(dst, minlength=N)
    assert deg.max() <= EW
    order = np.argsort(dst, kind="stable")
    dst_sorted = dst[order]

    core_wins = []  # per core: list of (node_base, span, estart, cnt)
    for c in range(NCORES):
        lo, hi = c * NPC, (c + 1) * NPC
        elo = int(np.searchsorted(dst_sorted, lo))
        wins = []
        node = lo
        eptr = elo
        while node < hi:
            base = node
            cnt = 0
            while node < hi and node - base < P:
                dn = int(deg[node])
                if cnt + dn > EW:
                    break
                cnt += dn
                node += 1
            wins.append((base, node - base, eptr, cnt))
            eptr += cnt
        core_wins.append(wins)

    NW = max(len(w) for w in core_wins)

    offs_all = []
    dstloc_all = []
    for c in range(NCORES):
        wins = core_wins[c]
        base_w = np.zeros(NW, np.int64)
        cnt_w = np.zeros(NW, np.int64)
        est_w = np.zeros(NW, np.int64)
        for i, (b, sp, es, cn) in enumerate(wins):
            base_w[i], est_w[i], cnt_w[i] = b, es, cn

        slot = np.arange(EW)  # slot = s*128 + p
        valid = slot[None, :] < cnt_w[:, None]  # [NW, EW]
        eidx = np.minimum(est_w[:, None] + slot[None, :], E - 1)
        eids = order[eidx]  # [NW, EW] edge ids (garbage where invalid)

        gidx = emi[eids].astype(np.int32)  # [NW, EW, 4]
        gidx[~valid] = 0
        # offs layout: [128 p, NW, NSUB, L]  (slot = s*128+p -> [s, p])
        offs = gidx.reshape(NW, NSUB, P, L).transpose(2, 0, 1, 3)
        offs_all.append(np.ascontiguousarray(offs, np.int32).reshape(P, NW * QW))

        dl = (dst[eids] - base_w[:, None]).astype(np.float32)  # [NW, EW]
        dl[~valid] = -1.0
        dloc = dl.reshape(NW, NSUB, P).transpose(2, 0, 1)  # [p, w, s]
        dstloc_all.append(np.ascontiguousarray(dloc, np.float32).reshape(P, NW * NSUB))

    return NW, core_wins, offs_all, dstloc_all


_CACHE = {}


def _build_program(NW):
    if NW in _CACHE:
        return _CACHE[NW]
    fp32 = mybir.dt.float32
    bf16 = mybir.dt.bfloat16
    nc = bacc.Bacc(
        "TRN2",
        target_bir_lowering=False,
        debug=not axon_active(),
        num_devices=NCORES,
    )
    feat_d = nc.dram_tensor("features", [N, D], fp32, kind="ExternalInput").ap()
    ab_d = nc.dram_tensor("ab", [P, 144], fp32, kind="ExternalInput").ap()
    offs_d = nc.dram_tensor("offs", [P, NW * QW], mybir.dt.int32, kind="ExternalInput").ap()
    dloc_d = nc.dram_tensor("dstloc", [P, NW * NSUB], fp32, kind="ExternalInput").ap()
    out_d = nc.dram_tensor("out", [NW * P, H * D], bf16, kind="ExternalOutput").ap()

    with tile.TileContext(nc) as tc:
        with (
            tc.tile_pool(name="const", bufs=1) as cpool,
            tc.tile_pool(name="oh", bufs=4) as ohpool,
            tc.tile_pool(name="ed", bufs=3) as edpool,
            tc.tile_pool(name="ts", bufs=6) as tspool,
            tc.tile_pool(name="sm", bufs=8) as smpool,
            tc.tile_pool(name="rhs", bufs=4) as rhspool,
            tc.tile_pool(name="ob", bufs=4) as obpool,
            tc.tile_pool(name="pt", bufs=3, space="PSUM") as ptpool,
            tc.tile_pool(name="pha", bufs=2, space="PSUM") as phapool,
            tc.tile_pool(name="pu", bufs=2, space="PSUM") as pupool,
            tc.tile_pool(name="pas", bufs=1, space="PSUM") as paspool,
        ):
            ident = cpool.tile([P, P], fp32, tag="ident")
            make_identity(nc, ident[:])
            iota = cpool.tile([P, P], fp32, tag="iota")
            nc.gpsimd.iota(
                iota[:], [[1, P]], channel_multiplier=0,
                allow_small_or_imprecise_dtypes=True,
            )
            ab_t = cpool.tile([P, 144], fp32, tag="ab")
            nc.sync.dma_start(out=ab_t[:], in_=ab_d[:, :])
            offs_t = cpool.tile([P, NW * QW], mybir.dt.int32, tag="offs")
            nc.sync.dma_start(out=offs_t[:], in_=offs_d[:, :])
            dloc_t = cpool.tile([P, NW * NSUB], fp32, tag="dloc")
            nc.sync.dma_start(out=dloc_t[:], in_=dloc_d[:, :])

            # ---- per-batch gather + compute pipeline
            for b in range((NW + GB - 1) // GB):
                gb = min(GB, NW - b * GB)
                ed_t = edpool.tile([P, GB * QW * D], fp32, tag="ed")
                for q in range(gb * QW):
                    nc.gpsimd.indirect_dma_start(
                        out=ed_t[:, q * D : (q + 1) * D],
                        out_offset=None,
                        in_=feat_d[:, :],
                        in_offset=bass.IndirectOffsetOnAxis(
                            ap=offs_t[:, b * GB * QW + q : b * GB * QW + q + 1],
                            axis=0,
                        ),
                    )
                for j in range(gb):
                    w = b * GB + j
                    u_p = pupool.tile([P, H * D], fp32, tag="u")
                    as_p = paspool.tile([P, H], fp32, tag="as")
                    for s in range(NSUB):
                        off = (j * QW + s * L) * D
                        ed_s = ed_t[:, off : off + L * D]  # [128, 256]
                        t0_p = ptpool.tile([P, P], fp32, tag="tp")
                        t1_p = ptpool.tile([P, P], fp32, tag="tp")
                        nc.tensor.transpose(out=t0_p[:], in_=ed_s[:, 0:P], identity=ident[:])
                        nc.tensor.transpose(out=t1_p[:], in_=ed_s[:, P : 2 * P], identity=ident[:])
                        t0_s = tspool.tile([P, P], fp32, tag="tsb")
                        t1_s = tspool.tile([P, P], fp32, tag="tsb")
                        nc.scalar.activation(out=t0_s[:], in_=t0_p[:], func=mybir.ActivationFunctionType.Copy)
                        nc.scalar.activation(out=t1_s[:], in_=t1_p[:], func=mybir.ActivationFunctionType.Copy)
                        ha_p = phapool.tile([P, 72], fp32, tag="ha")
                        nc.tensor.matmul(out=ha_p[:], lhsT=t0_s[:], rhs=ab_t[:, 0:72], start=True, stop=False)
                        nc.tensor.matmul(out=ha_p[:], lhsT=t1_s[:], rhs=ab_t[:, 72:144], start=False, stop=True)
                        # exp(leaky_relu(a)) on ACT (both in the exp act table set)
                        alr = smpool.tile([P, H], fp32, tag="alr")
                        nc.scalar.activation(
                            out=alr[:], in_=ha_p[:, D : D + H],
                            func=mybir.ActivationFunctionType.Prelu, alpha=ALPHA,
                        )
                        aexp = smpool.tile([P, H], bf16, tag="aexp")
                        nc.scalar.activation(out=aexp[:], in_=alr[:], func=mybir.ActivationFunctionType.Exp)
                        # one-hot scatter matrix [slot, node] built on DVE
                        oh_t = ohpool.tile([P, P], bf16, tag="oh")
                        nc.vector.tensor_scalar(
                            out=oh_t[:], in0=iota[:],
                            scalar1=dloc_t[:, w * NSUB + s : w * NSUB + s + 1],
                            scalar2=None,
                            op0=mybir.AluOpType.is_equal,
                        )
                        # rhs [128, 512] = hidden (h-bcast) * aexp (d-bcast)
                        rhs_t = rhspool.tile([P, H * D], bf16, tag="rhs")
                        nc.vector.tensor_tensor(
                            out=rhs_t[:].rearrange("p (h d) -> p h d", h=H),
                            in0=ha_p[:, 0:D].unsqueeze(1).to_broadcast([P, H, D]),
                            in1=aexp[:].unsqueeze(2).to_broadcast([P, H, D]),
                            op=mybir.AluOpType.mult,
                        )
                        nc.tensor.matmul(
                            out=u_p[:], lhsT=oh_t[:], rhs=rhs_t[:],
                            start=(s == 0), stop=(s == NSUB - 1),
                            skip_group_check=True,
                        )
                        nc.tensor.matmul(
                            out=as_p[:], lhsT=oh_t[:], rhs=aexp[:],
                            start=(s == 0), stop=(s == NSUB - 1),
                            skip_group_check=True,
                        )
                    # epilogue: out = U / max(asum, eps), cast to bf16
                    asafe = smpool.tile([P, H], fp32, tag="asafe")
                    nc.vector.tensor_scalar_max(out=asafe[:], in0=as_p[:], scalar1=1e-20)
                    rec = smpool.tile([P, H], fp32, tag="rec")
                    nc.vector.reciprocal(out=rec[:], in_=asafe[:])
                    o_sb = obpool.tile([P, H * D], bf16, tag="osb")
                    nc.vector.tensor_tensor(
                        out=o_sb[:].rearrange("p (h d) -> p h d", h=H),
                        in0=u_p[:].rearrange("p (h d) -> p h d", h=H),
                        in1=rec[:].unsqueeze(2).to_broadcast([P, H, D]),
                        op=mybir.AluOpType.mult,
                    )
                    nc.sync.dma_start(out=out_d[w * P : (w + 1) * P, :], in_=o_sb[:])

    nc.compile()
    _CACHE[NW] = nc
    return nc


LAST_RESULT = None


def kernel(features, r_vec, attn1_w, attn2, edge_metapath_indices, edge_dst):
    global LAST_RESULT
    features = np.ascontiguousarray(np.asarray(features, np.float32))
    A, B = _build_AB(r_vec, attn1_w, attn2)
    ab = np.concatenate([A, B], axis=1)  # [256, 72]
    # ab dram layout [128, 144]: [:, 0:72] = rows 0:128, [:, 72:144] = rows 128:256
    ab_host = np.concatenate([ab[:P], ab[P:]], axis=1).astype(np.float32)
    ab_host = np.ascontiguousarray(ab_host)

    NW, core_wins, offs_all, dstloc_all = _host_prep(
        edge_metapath_indices, edge_dst
    )
    nc = _build_program(NW)

    in_maps = []
    for c in range(NCORES):
        in_maps.append(
            {
                "features": features,
                "ab": ab_host,
                "offs": offs_all[c],
                "dstloc": dstloc_all[c],
            }
        )
    res = run_bass_kernel_spmd(nc, in_maps, core_ids=list(range(NCORES)))
    LAST_RESULT = res

    out = np.zeros((N, H, D), np.float32)
    for c in range(NCORES):
        co = np.asarray(res.results[c]["out"], dtype=np.float32)  # [NW*128, 512]
        for w, (base, span, _es, _cn) in enumerate(core_wins[c]):
            if span > 0:
                out[base : base + span] = co[w * P : w * P + span].reshape(span, H, D)
    return out


if __name__ == "__main__":
    import reference

    inputs = {k: np.asarray(v) for k, v in reference.setup_inputs().items()}
    got = kernel(**inputs)
    print("kernel output", got.shape, got.dtype)
